# revision 48
# baseline (speedup 1.0000x reference)
"""Multi-head attention TRN2 kernel (b=4, n=4096, e=128, h=4, d=32).

Sharding: 16 (batch, query-half) units over 8 cores; core c handles batch
c//2, query rows (c%2)*2048..+2048.  Each core computes q/k/v projections
for its batch (k,v over all 4096 keys), 4 attention heads over its 2048
query rows, and the output projection for those rows.

On-device pipeline (transpose-free):
  scoresT[k,q] = matmul(lhsT=kT_h, rhs=qT_h)        4-way PE row-tiles
  expT: split between ScalarE (table Exp -> bf16) and VectorE
        (Schraudolph: int16(s*128/ln2 + 16251.3) bitcast bf16)
  attT[[v|1]] = matmul(lhsT=[v_h|1], rhs=expT)      row 32 = denominator,
        2-way PE col-tiles (offsets 0/64); no separate sum matmuls
  rinv broadcast via gpsimd partition_broadcast; DVE normalize -> bf16
  out[q,e] = matmul(lhsT=attnT, rhs=W_proj) + bias
"""

import os
import sys

sys.path.insert(0, "/opt/trn_rl_repo")
os.environ.setdefault("NEURON_RT_RESET_CORES", "1")

import numpy as np
import ml_dtypes

E, H, D = 128, 4, 32
B, N = 4, 4096
NCORES = 8
NQ = N // 2  # per-core query rows
QB = 512  # query block
NKB = N // 128  # 32 key chunks
SCALE = float(1.0 / np.sqrt(np.float32(E)))
# Schraudolph exp in bf16: bf16(bits = int16(s*SCALE*128/ln2 + B)) ~= exp(s*SCALE)
SCHR_A = float(SCALE * 128.0 / np.log(2.0))
SCHR_B = 127.0 * 128.0 - 4.7

def use_dve_for(jj):
    """exp engine schedule: p0 always ACT; p1 on DVE for 3 of 4 units
    (37.5%% of units overall, never both halves of a unit on DVE)."""
    return (jj % 2 == 1) and ((jj // 2) % 4 != 0)

_CACHE = {}


def _split_multi_waits(nc):
    """This neuronxcc build accepts at most ONE sync wait per instruction;
    Tile emits up to two.  Hoist extra waits onto same-engine NoOps."""
    from concourse import mybir as mb

    for fn in nc.m.functions:
        for blk in fn.blocks:
            insts = list(blk.instructions)
            if not any(
                i.sync_info and i.sync_info.on_wait and len(i.sync_info.on_wait) > 1
                for i in insts
            ):
                continue
            new = []
            for inst in insts:
                si = inst.sync_info
                if si is not None and si.on_wait and len(si.on_wait) > 1:
                    waits = list(si.on_wait)
                    for j, w in enumerate(waits[:-1]):
                        new.append(
                            mb.InstNoOp(
                                name=f"{inst.name}-wsplit{j}",
                                engine=inst.engine,
                                ins=[],
                                outs=[],
                                sync_info=mb.SyncInfo(on_wait=[w], on_update=[]),
                            )
                        )
                    inst.sync_info = mb.SyncInfo(
                        on_wait=[waits[-1]], on_update=list(si.on_update or [])
                    )
                new.append(inst)
            blk.instructions = new


def _build(split=True):
    import concourse.bass as bass
    import concourse.tile as tile
    from concourse import mybir
    from concourse.vector_clock import ScopedClock, VectorClock

    f32 = mybir.dt.float32
    bf16 = mybir.dt.bfloat16
    i16 = mybir.dt.int16

    class SplitDrainTileContext(tile.TileContext):
        """Final drain waits one-sem-per-instruction (walrus limit)."""

        def _drain_and_barrier(self, tick_clock, wait_clock):
            vc = tick_clock.global_clock
            n = len(vc)
            for p in range(n):
                t = vc[p]
                if t <= 0:
                    continue
                pvec = [0] * n
                pvec[p] = t
                nop_inst = self.nc.sync.nop()
                wait_clock.add_sem_waits(
                    nop_inst.ins, ScopedClock({None: VectorClock(pvec)})
                )
            self.nc.sync.drain()
            self.nc.all_engine_barrier()
            assert self.sems is not None
            popped = self.nc._tile_sem_poison_stack.pop()
            assert popped is self._sem_poison
            self.nc.clear_and_free_semaphores(list(self.sems.allocated().values()))
            self.nc.all_engine_barrier()

    nc = bass.Bass("TRN2", target_bir_lowering=False, debug=False, num_devices=NCORES)

    xT_kv = nc.dram_tensor("xT_kv", [E, N], bf16, kind="ExternalInput")
    xT_q = nc.dram_tensor("xT_q", [E, NQ], bf16, kind="ExternalInput")
    # wpack cols: Wq 0:128 | Wk 128:256 | Wv 256:384 | Wp 384:512 | bq 512 | bk 513
    wpack = nc.dram_tensor("wpack", [E, 4 * E + 2], f32, kind="ExternalInput")
    bp = nc.dram_tensor("bp", [1, E], f32, kind="ExternalInput")
    out = nc.dram_tensor("out", [NQ, E], f32, kind="ExternalOutput")
    # scratch for the rinv partition-broadcast round-trip (qb parity halves)
    rscr = nc.dram_tensor("rscr", [2, 2, 2 * QB], f32, kind="Internal")
    rscr2 = nc.dram_tensor("rscr2", [2, H * QB], bf16, kind="Internal")

    with SplitDrainTileContext(nc) as tc:
        import contextlib

        with contextlib.ExitStack() as ctx:
            consts = ctx.enter_context(tc.tile_pool(name="consts", bufs=1))
            data = ctx.enter_context(tc.tile_pool(name="data", bufs=1))
            expool = ctx.enter_context(tc.tile_pool(name="expool", bufs=6))
            nrm = ctx.enter_context(tc.tile_pool(name="nrm", bufs=2))
            outp = ctx.enter_context(tc.tile_pool(name="outp", bufs=2))

            # ---- input DMAs: weights first (gate the first matmuls),
            # xkv on a separate queue so wires overlap ----
            wp_f = consts.tile([E, 4 * E + 2], f32)
            nc.gpsimd.dma_start(out=wp_f[:], in_=wpack[:])
            xq_s = data.tile([E, NQ], bf16)
            nc.gpsimd.dma_start(out=xq_s[:], in_=xT_q[:])
            xkv_s = data.tile([E, N], bf16)
            nc.sync.dma_start(out=xkv_s[:, :2048], in_=xT_kv[:, :2048])
            nc.sync.dma_start(out=xkv_s[:, 2048:], in_=xT_kv[:, 2048:])
            bp_s = consts.tile([E, E], f32)
            bp_bcast = bass.AP(
                tensor=bp.ap().tensor,
                offset=bp.ap().offset,
                ap=[[0, E], [1, E]],
            )
            nc.gpsimd.dma_start(out=bp_s[:], in_=bp_bcast)

            # weights cast to bf16 (projection matmuls run bf16)
            wqkvp = consts.tile([E, 4 * E], bf16)
            nc.vector.tensor_copy(wqkvp[:], wp_f[:, : 4 * E])
            wq_s = wqkvp[:, 0 * E : 1 * E]
            wk_s = wqkvp[:, 1 * E : 2 * E]
            wv_s = wqkvp[:, 2 * E : 3 * E]
            wps_s = wqkvp[:, 3 * E : 4 * E]
            bq_s = wp_f[:, 4 * E : 4 * E + 1]
            bk_s = wp_f[:, 4 * E + 1 : 4 * E + 2]

            # persistent denominator-gather tiles (gap rows stay 1.0)
            r4ab = [data.tile([E, QB], f32, name=f"r4{i}") for i in range(2)]
            for t in r4ab:
                nc.vector.memset(t[0 : 3 * D + 1, :], 1.0)

            qT = data.tile([E, NQ], bf16)  # [(h d), q] with q-bias added
            kT = data.tile([E, N], bf16)  # [(h d), k] with k-bias added
            # v with ones column: [keys=128, chunk, head, 33]; col 32 == 1.0
            v1 = data.tile([E, NKB, H, D + 1], bf16)
            nc.vector.memset(v1[:, :, :, D], 1.0)

            pssc = ctx.enter_context(tc.tile_pool(name="pssc", bufs=3, space="PSUM"))
            psatt = ctx.enter_context(tc.tile_pool(name="psatt", bufs=1, space="PSUM"))

            # ---- qkv projections (bf16 matmuls) ----
            def emit_qT():
                for j in range(0, NQ, QB):
                    ps = pssc.tile([E, 2 * QB], f32, tag="scps", name=f"qps{j}")
                    nc.tensor.matmul(
                        ps[:, :QB], wq_s, xq_s[:, j : j + QB], start=True, stop=True
                    )
                    nc.vector.tensor_scalar_add(qT[:, j : j + QB], ps[:, :QB], bq_s)

            def emit_kT_chunk(c):
                j = c * QB
                ps = pssc.tile([E, 2 * QB], f32, tag="scps", name=f"kps{j}")
                nc.tensor.matmul(
                    ps[:, :QB], wk_s, xkv_s[:, j : j + QB], start=True, stop=True
                )
                nc.vector.tensor_scalar_add(kT[:, j : j + QB], ps[:, :QB], bk_s)

            def emit_v_chunk(m):
                ps = pssc.tile([E, 2 * QB], f32, tag="scps", name=f"vps{m}")
                nc.tensor.matmul(
                    ps[:, :E],
                    xkv_s[:, 128 * m : 128 * m + 128],
                    wv_s,
                    start=True,
                    stop=True,
                )
                # one strided copy: psum [128,(h,d)] -> v1[:, m, h, 0:32]
                nc.vector.tensor_copy(
                    v1[:, m, :, 0:D],
                    ps[:, :E].rearrange("p (h d) -> p h d", h=H),
                )

            emit_qT()
            emit_kT_chunk(0)

            # ---- attention ----
            # att psum layout: head h -> partitions 64*(h%2) + [0,33),
            #                  free offset (h//2)*QB
            def att_slice(att_ps, h, rows=slice(0, D)):
                p0 = 64 * (h % 2)
                f0 = (h // 2) * QB
                return att_ps[
                    p0 + rows.start : p0 + rows.stop, f0 : f0 + QB
                ]

            ROW_OF_HEAD = {0: 0, 1: 1, 2: 2, 3: 3}

            def emit_norm_a(qb, att_ps):
                """the only att-psum readers: ACT gathers denominators,
                DVE copies raw att -> SBUF bf16.  Frees att psum fast."""
                r4 = r4ab[qb % 2]
                for h in range(H):
                    p0 = 64 * (h % 2) + D
                    f0 = (h // 2) * QB
                    nc.scalar.activation(
                        out=r4[D * h : D * h + 1, :],
                        in_=att_ps[p0 : p0 + 1, f0 : f0 + QB],
                        func=mybir.ActivationFunctionType.Copy,
                    )
                attnT = nrm.tile([E, QB], bf16, tag="attnT", name=f"attnT{qb}")
                for h in range(H):
                    nc.vector.tensor_copy(
                        attnT[D * h : D * h + D, :], att_slice(att_ps, h)
                    )
                return attnT

            def emit_norm_b(qb, attnT):
                """reciprocal + DRAM round-trip broadcast + Pool normalize"""
                half = qb % 2
                r4 = r4ab[half]
                rinv4 = nrm.tile([E, QB], bf16, tag="rinv4", name=f"ri{qb}")
                with nc.allow_low_precision(reason="softmax rinv in bf16 is ample"):
                    nc.vector.reciprocal(
                        rinv4[0 : 3 * D + 1, :], r4[0 : 3 * D + 1, :]
                    )
                scr2 = rscr2.ap()
                for h in range(H):
                    ri_d = bass.AP(
                        tensor=scr2.tensor,
                        offset=scr2.offset + (half * H + h) * QB,
                        ap=[[1, 1], [1, QB]],
                    )
                    nc.sync.dma_start(
                        out=ri_d, in_=rinv4[D * h : D * h + 1, :]
                    )
                rbc = nrm.tile([E, QB], bf16, tag="rbc", name=f"rbc{qb}")
                for h in range(H):
                    ri_b = bass.AP(
                        tensor=scr2.tensor,
                        offset=scr2.offset + (half * H + ROW_OF_HEAD[h]) * QB,
                        ap=[[0, D], [1, QB]],
                    )
                    nc.sync.dma_start(out=rbc[D * h : D * h + D, :], in_=ri_b)
                attn2 = nrm.tile([E, QB], bf16, tag="attn2", name=f"attn2_{qb}")
                with nc.allow_low_precision(reason="softmax normalize in bf16"):
                    for h in range(H):
                        nc.gpsimd.tensor_tensor(
                            attn2[D * h : D * h + D, :],
                            attnT[D * h : D * h + D, :],
                            rbc[D * h : D * h + D, :],
                            op=mybir.AluOpType.mult,
                        )
                return attn2

            def emit_proj(qb, attnT):
                """project + store query block qb"""
                q0 = qb * QB
                pp = pssc.tile([E, 2 * QB], f32, tag="scps", name=f"pp{qb}")
                for m in range(QB // 128):
                    nc.tensor.matmul(
                        pp[:, 128 * m : 128 * m + 128],
                        attnT[:, 128 * m : 128 * m + 128],
                        wps_s,
                        start=(m == 0),
                        stop=(m == QB // 128 - 1),
                        skip_group_check=True,
                    )
                ob = outp.tile([E, QB], f32, tag="ob", name=f"ob{qb}")
                bp_rep = bass.AP(
                    tensor=bp_s[:].tensor,
                    offset=bp_s[:].offset,
                    ap=[list(bp_s[:].ap[0]), [0, QB // 128], [1, E]],
                )
                ob_v = ob[:].rearrange("p (m e) -> p m e", e=E)
                pp_v = pp[:, :QB].rearrange("p (m e) -> p m e", e=E)
                nc.vector.tensor_add(ob_v, pp_v, bp_rep)
                # single 3D DMA: ob [q=128, m, e] -> out rows q0+128m+p
                ob_dram = bass.AP(
                    tensor=out.ap().tensor,
                    offset=out.ap().offset + q0 * E,
                    ap=[[E, 128], [128 * E, QB // 128], [1, E]],
                )
                nc.sync.dma_start(out=ob_dram, in_=ob[:].rearrange("p (m e) -> p m e", e=E))

            NSC = NQ // QB * NKB * 2  # 256 (qb, chunk, pair) units

            def sc_tile(jj):
                return pssc.tile([E, 2 * QB], f32, tag="scps", name=f"sc{jj}")

            def emit_sc(jj, sc):
                qb, c, p = jj // (2 * NKB), (jj % (2 * NKB)) // 2, jj % 2
                q0, k0 = qb * QB, 128 * c
                for hh in range(2):
                    h = 2 * p + hh
                    nc.tensor.matmul(
                        sc[:, QB * hh : QB * hh + QB],
                        kT[D * h : D * h + D, k0 : k0 + 128],
                        qT[D * h : D * h + D, q0 : q0 + QB],
                        start=True,
                        stop=True,
                        tile_position=(D * h, 0),
                    )

            def emit_exp(jj, sc):
                """one exp unit: sc [E,1024] -> ex (bf16 view)"""
                use_dve = use_dve_for(jj)
                if use_dve:
                    ex = expool.tile([E, 2 * QB], i16, tag="ex", name=f"ex{jj}")
                    nc.vector.tensor_scalar(
                        ex[:],
                        sc[:],
                        SCHR_A,
                        SCHR_B,
                        op0=mybir.AluOpType.mult,
                        op1=mybir.AluOpType.add,
                    )
                    return ex[:].bitcast(mybir.dt.bfloat16)
                ex = expool.tile(
                    [E, 2 * QB], mybir.dt.bfloat16, tag="ex", name=f"ex{jj}"
                )
                nc.scalar.activation(
                    out=ex[:],
                    in_=sc[:],
                    func=mybir.ActivationFunctionType.Exp,
                    scale=SCALE,
                )
                return ex[:]

            def emit_pv(att_ps, c, exvs):
                for h in range(H):
                    nc.tensor.matmul(
                        att_slice(att_ps, h, slice(0, D + 1)),
                        v1[:, c, h, :],
                        exvs[h // 2][:, QB * (h % 2) : QB * (h % 2) + QB],
                        start=(c == 0),
                        stop=(c == NKB - 1),
                        tile_position=(0, 64 * (h % 2)),
                        skip_group_check=True,
                    )

            # per (qb, c): 4 score matmuls (chunk c+1), 2 exp units (chunk c),
            # 4 PV matmuls (chunk c) -- large same-kind PE groups
            NCU = NQ // QB * NKB  # 128 (qb, chunk) units
            att_tiles = {}
            pending_proj = []
            scs = {}

            def emit_scores(u):
                qb, c = u // NKB, u % NKB
                for p in range(2):
                    jj = 2 * u + p
                    scs[jj] = sc_tile(jj)
                    emit_sc(jj, scs[jj])

            # PV lags scores by one unit: PE never waits on the current
            # unit's exp, only the previous one's (already drained)
            emit_scores(0)
            exq = {}  # u -> exvs
            for u in range(NCU + 1):
                if u < NCU:
                    qb, c = u // NKB, u % NKB
                    if qb == 0:
                        # stream the kv projections under the first query block
                        if c % 4 == 0 and (c // 4 + 1) < N // QB:
                            emit_kT_chunk(c // 4 + 1)
                        emit_v_chunk(c)
                    if u + 1 < NCU:
                        emit_scores(u + 1)
                up = u - 1
                boundary = u >= 1 and (up % NKB) == NKB - 1
                if u < NCU and not boundary:
                    exq[u] = [emit_exp(2 * u + p, scs.pop(2 * u + p)) for p in range(2)]
                if u >= 1:
                    pqb, pc = up // NKB, up % NKB
                    if pc == 0:
                        att_tiles[pqb] = psatt.tile(
                            [E, 2 * QB], f32, tag="attps", name=f"attp{pqb}"
                        )
                    emit_pv(att_tiles[pqb], pc, exq.pop(up))
                    if boundary:
                        attnT = emit_norm_a(pqb, att_tiles.pop(pqb))
                        if u < NCU:
                            exq[u] = [
                                emit_exp(2 * u + p, scs.pop(2 * u + p))
                                for p in range(2)
                            ]
                        pending_proj.append((pqb, emit_norm_b(pqb, attnT)))
                    if pc == 8 and pending_proj:
                        piq, pattnT = pending_proj.pop(0)
                        emit_proj(piq, pattnT)
            while pending_proj:
                piq, pattnT = pending_proj.pop(0)
                emit_proj(piq, pattnT)

    if split:
        _split_multi_waits(nc)
    return nc


def _prep_host(x, W_qkv, b_qkv, W_proj, b_proj):
    j = np.arange(E)
    h, d = j // D, j % D
    cq = h * (3 * D) + d * 3 + 0
    ck = cq + 1
    cv = cq + 2
    Wq = W_qkv[:, cq].astype(np.float32)
    Wk = W_qkv[:, ck].astype(np.float32)
    Wv = W_qkv[:, cv].astype(np.float32)
    bq = b_qkv[cq].astype(np.float32)
    bk = b_qkv[ck].astype(np.float32)
    bv = b_qkv[cv].astype(np.float32)
    bp = (bv @ W_proj + b_proj).astype(np.float32).reshape(1, E)
    wpack = np.concatenate(
        [Wq, Wk, Wv, W_proj.astype(np.float32), bq.reshape(E, 1), bk.reshape(E, 1)],
        axis=1,
    )
    wpack = np.ascontiguousarray(wpack, np.float32)
    in_maps = []
    for c in range(NCORES):
        b, half = c // 2, c % 2
        xT_kv = np.ascontiguousarray(x[b].T).astype(ml_dtypes.bfloat16)
        xT_q = np.ascontiguousarray(
            x[b, half * NQ : (half + 1) * NQ].T
        ).astype(ml_dtypes.bfloat16)
        in_maps.append(
            {"xT_kv": xT_kv, "xT_q": xT_q, "wpack": wpack, "bp": bp}
        )
    return in_maps


def kernel(x, W_qkv, b_qkv, W_proj, b_proj, _trace=False):
    x = np.asarray(x, np.float32)
    W_qkv = np.asarray(W_qkv, np.float32)
    b_qkv = np.asarray(b_qkv, np.float32)
    W_proj = np.asarray(W_proj, np.float32)
    b_proj = np.asarray(b_proj, np.float32)

    from concourse.bass_utils import run_bass_kernel_spmd

    if "nc" not in _CACHE:
        _CACHE["nc"] = _build()
    nc = _CACHE["nc"]

    in_maps = _prep_host(x, W_qkv, b_qkv, W_proj, b_proj)
    res = run_bass_kernel_spmd(
        nc, in_maps, core_ids=list(range(NCORES)), trace=_trace
    )
    out = np.empty((B, N, E), np.float32)
    for c in range(NCORES):
        b, half = c // 2, c % 2
        out[b, half * NQ : (half + 1) * NQ] = res.results[c]["out"]
    if _trace:
        _CACHE["last_result"] = res
    return out


# revision 49
# speedup vs baseline: 1.0582x; 1.0582x over previous
"""Multi-head attention TRN2 kernel (b=4, n=4096, e=128, h=4, d=32).

Sharding: 16 (batch, query-half) units over 8 cores; core c handles batch
c//2, query rows (c%2)*2048..+2048.  Each core computes q/k/v projections
for its batch (k,v over all 4096 keys), 4 attention heads over its 2048
query rows, and the output projection for those rows.

On-device pipeline (transpose-free):
  scoresT[k,q] = matmul(lhsT=kT_h, rhs=qT_h)        4-way PE row-tiles
  expT: split between ScalarE (table Exp -> bf16) and VectorE
        (Schraudolph: int16(s*128/ln2 + 16251.3) bitcast bf16)
  attT[[v|1]] = matmul(lhsT=[v_h|1], rhs=expT)      row 32 = denominator,
        2-way PE col-tiles (offsets 0/64); no separate sum matmuls
  rinv broadcast via gpsimd partition_broadcast; DVE normalize -> bf16
  out[q,e] = matmul(lhsT=attnT, rhs=W_proj) + bias
"""

import os
import sys

sys.path.insert(0, "/opt/trn_rl_repo")
os.environ.setdefault("NEURON_RT_RESET_CORES", "1")

import numpy as np
import ml_dtypes

E, H, D = 128, 4, 32
B, N = 4, 4096
NCORES = 8
NQ = N // 2  # per-core query rows
QB = 512  # query block
NKB = N // 128  # 32 key chunks
SCALE = float(1.0 / np.sqrt(np.float32(E)))
# Schraudolph exp in bf16: bf16(bits = int16(s*SCALE*128/ln2 + B)) ~= exp(s*SCALE)
SCHR_A = float(SCALE * 128.0 / np.log(2.0))
SCHR_B = 127.0 * 128.0 - 4.7

def use_dve_for(jj):
    """exp engine schedule: p0 always ACT; p1 on DVE for 3 of 4 units
    (37.5%% of units overall, never both halves of a unit on DVE)."""
    return (jj % 2 == 1) and ((jj // 2) % 4 != 0)

_CACHE = {}


def _split_multi_waits(nc):
    """This neuronxcc build accepts at most ONE sync wait per instruction;
    Tile emits up to two.  Hoist extra waits onto same-engine NoOps."""
    from concourse import mybir as mb

    for fn in nc.m.functions:
        for blk in fn.blocks:
            insts = list(blk.instructions)
            if not any(
                i.sync_info and i.sync_info.on_wait and len(i.sync_info.on_wait) > 1
                for i in insts
            ):
                continue
            new = []
            for inst in insts:
                si = inst.sync_info
                if si is not None and si.on_wait and len(si.on_wait) > 1:
                    waits = list(si.on_wait)
                    for j, w in enumerate(waits[:-1]):
                        new.append(
                            mb.InstNoOp(
                                name=f"{inst.name}-wsplit{j}",
                                engine=inst.engine,
                                ins=[],
                                outs=[],
                                sync_info=mb.SyncInfo(on_wait=[w], on_update=[]),
                            )
                        )
                    inst.sync_info = mb.SyncInfo(
                        on_wait=[waits[-1]], on_update=list(si.on_update or [])
                    )
                new.append(inst)
            blk.instructions = new


def _build(split=True):
    import concourse.bass as bass
    import concourse.tile as tile
    from concourse import mybir
    from concourse.vector_clock import ScopedClock, VectorClock

    f32 = mybir.dt.float32
    bf16 = mybir.dt.bfloat16
    i16 = mybir.dt.int16

    class SplitDrainTileContext(tile.TileContext):
        """Final drain waits one-sem-per-instruction (walrus limit)."""

        def _drain_and_barrier(self, tick_clock, wait_clock):
            vc = tick_clock.global_clock
            n = len(vc)
            for p in range(n):
                t = vc[p]
                if t <= 0:
                    continue
                pvec = [0] * n
                pvec[p] = t
                nop_inst = self.nc.sync.nop()
                wait_clock.add_sem_waits(
                    nop_inst.ins, ScopedClock({None: VectorClock(pvec)})
                )
            self.nc.sync.drain()
            self.nc.all_engine_barrier()
            assert self.sems is not None
            popped = self.nc._tile_sem_poison_stack.pop()
            assert popped is self._sem_poison
            self.nc.clear_and_free_semaphores(list(self.sems.allocated().values()))
            self.nc.all_engine_barrier()

    nc = bass.Bass("TRN2", target_bir_lowering=False, debug=False, num_devices=NCORES)

    xT_kv = nc.dram_tensor("xT_kv", [E, N], bf16, kind="ExternalInput")
    xT_q = nc.dram_tensor("xT_q", [E, NQ], bf16, kind="ExternalInput")
    # wpack cols: Wq 0:128 | Wk 128:256 | Wv 256:384 | Wp 384:512 | bq 512 | bk 513
    wpack = nc.dram_tensor("wpack", [E, 4 * E + 2], f32, kind="ExternalInput")
    bp = nc.dram_tensor("bp", [1, E], f32, kind="ExternalInput")
    out = nc.dram_tensor("out", [NQ, E], f32, kind="ExternalOutput")
    # scratch for the rinv partition-broadcast round-trip (qb parity halves)
    rscr = nc.dram_tensor("rscr", [2, 2, 2 * QB], f32, kind="Internal")
    rscr2 = nc.dram_tensor("rscr2", [2, H * QB], bf16, kind="Internal")

    with SplitDrainTileContext(nc) as tc:
        import contextlib

        with contextlib.ExitStack() as ctx:
            consts = ctx.enter_context(tc.tile_pool(name="consts", bufs=1))
            data = ctx.enter_context(tc.tile_pool(name="data", bufs=1))
            expool = ctx.enter_context(tc.tile_pool(name="expool", bufs=6))
            nrm = ctx.enter_context(tc.tile_pool(name="nrm", bufs=2))
            outp = ctx.enter_context(tc.tile_pool(name="outp", bufs=2))

            # ---- input DMAs: weights first (gate the first matmuls),
            # xkv on a separate queue so wires overlap ----
            wp_f = consts.tile([E, 4 * E + 2], f32)
            nc.gpsimd.dma_start(out=wp_f[:], in_=wpack[:])
            xq_s = data.tile([E, NQ], bf16)
            nc.gpsimd.dma_start(out=xq_s[:], in_=xT_q[:])
            xkv_s = data.tile([E, N], bf16)
            nc.sync.dma_start(out=xkv_s[:, :2048], in_=xT_kv[:, :2048])
            nc.sync.dma_start(out=xkv_s[:, 2048:], in_=xT_kv[:, 2048:])
            bp_s = consts.tile([E, E], f32)
            bp_bcast = bass.AP(
                tensor=bp.ap().tensor,
                offset=bp.ap().offset,
                ap=[[0, E], [1, E]],
            )
            nc.gpsimd.dma_start(out=bp_s[:], in_=bp_bcast)

            # weights cast to bf16 (projection matmuls run bf16)
            wqkvp = consts.tile([E, 4 * E], bf16)
            nc.vector.tensor_copy(wqkvp[:], wp_f[:, : 4 * E])
            wq_s = wqkvp[:, 0 * E : 1 * E]
            wk_s = wqkvp[:, 1 * E : 2 * E]
            wv_s = wqkvp[:, 2 * E : 3 * E]
            wps_s = wqkvp[:, 3 * E : 4 * E]
            bq_s = wp_f[:, 4 * E : 4 * E + 1]
            bk_s = wp_f[:, 4 * E + 1 : 4 * E + 2]

            # persistent denominator-gather tiles (gap rows stay 1.0)
            r4ab = [data.tile([E, QB], f32, name=f"r4{i}") for i in range(2)]
            for t in r4ab:
                nc.vector.memset(t[0 : 3 * D + 1, :], 1.0)

            qT = data.tile([E, NQ], bf16)  # [(h d), q] with q-bias added
            kT = data.tile([E, N], bf16)  # [(h d), k] with k-bias added
            # v with ones column: [keys=128, chunk, head, 33]; col 32 == 1.0
            v1 = data.tile([E, NKB, H, D + 1], bf16)
            nc.vector.memset(v1[:, :, :, D], 1.0)

            pssc = ctx.enter_context(tc.tile_pool(name="pssc", bufs=3, space="PSUM"))
            psatt = ctx.enter_context(tc.tile_pool(name="psatt", bufs=1, space="PSUM"))

            # ---- qkv projections (bf16 matmuls) ----
            def emit_qT():
                for j in range(0, NQ, QB):
                    ps = pssc.tile([E, 2 * QB], f32, tag="scps", name=f"qps{j}")
                    nc.tensor.matmul(
                        ps[:, :QB], wq_s, xq_s[:, j : j + QB], start=True, stop=True
                    )
                    nc.vector.tensor_scalar_add(qT[:, j : j + QB], ps[:, :QB], bq_s)

            def emit_kT_chunk(c):
                j = c * QB
                ps = pssc.tile([E, 2 * QB], f32, tag="scps", name=f"kps{j}")
                nc.tensor.matmul(
                    ps[:, :QB], wk_s, xkv_s[:, j : j + QB], start=True, stop=True
                )
                nc.vector.tensor_scalar_add(kT[:, j : j + QB], ps[:, :QB], bk_s)

            def emit_v_chunk(m):
                ps = pssc.tile([E, 2 * QB], f32, tag="scps", name=f"vps{m}")
                nc.tensor.matmul(
                    ps[:, :E],
                    xkv_s[:, 128 * m : 128 * m + 128],
                    wv_s,
                    start=True,
                    stop=True,
                )
                # one strided copy: psum [128,(h,d)] -> v1[:, m, h, 0:32]
                nc.vector.tensor_copy(
                    v1[:, m, :, 0:D],
                    ps[:, :E].rearrange("p (h d) -> p h d", h=H),
                )

            emit_qT()
            emit_kT_chunk(0)

            # ---- attention ----
            # att psum layout: head h -> partitions 64*(h%2) + [0,33),
            #                  free offset (h//2)*QB
            def att_slice(att_ps, h, rows=slice(0, D)):
                p0 = 64 * (h % 2)
                f0 = (h // 2) * QB
                return att_ps[
                    p0 + rows.start : p0 + rows.stop, f0 : f0 + QB
                ]

            ROW_OF_HEAD = {0: 0, 1: 1, 2: 2, 3: 3}

            def emit_norm_a(qb, att_ps):
                """the only att-psum readers: DVE gathers denominators (so
                the scheduler puts the reciprocal right after them), ACT
                copies raw att -> SBUF bf16.  Frees att psum fast."""
                r4 = r4ab[qb % 2]
                for h in range(H):
                    p0 = 64 * (h % 2) + D
                    f0 = (h // 2) * QB
                    nc.vector.tensor_copy(
                        r4[D * h : D * h + 1, :], att_ps[p0 : p0 + 1, f0 : f0 + QB]
                    )
                attnT = nrm.tile([E, QB], bf16, tag="attnT", name=f"attnT{qb}")
                for h in range(H):
                    nc.scalar.activation(
                        out=attnT[D * h : D * h + D, :],
                        in_=att_slice(att_ps, h),
                        func=mybir.ActivationFunctionType.Copy,
                    )
                return attnT

            def emit_norm_b(qb, attnT):
                """reciprocal + DRAM round-trip broadcast + Pool normalize"""
                half = qb % 2
                r4 = r4ab[half]
                rinv4 = nrm.tile([E, QB], bf16, tag="rinv4", name=f"ri{qb}")
                with nc.allow_low_precision(reason="softmax rinv in bf16 is ample"):
                    nc.vector.reciprocal(
                        rinv4[0 : 3 * D + 1, :], r4[0 : 3 * D + 1, :]
                    )
                scr2 = rscr2.ap()
                for h in range(H):
                    ri_d = bass.AP(
                        tensor=scr2.tensor,
                        offset=scr2.offset + (half * H + h) * QB,
                        ap=[[1, 1], [1, QB]],
                    )
                    nc.sync.dma_start(
                        out=ri_d, in_=rinv4[D * h : D * h + 1, :]
                    )
                rbc = nrm.tile([E, QB], bf16, tag="rbc", name=f"rbc{qb}")
                for h in range(H):
                    ri_b = bass.AP(
                        tensor=scr2.tensor,
                        offset=scr2.offset + (half * H + ROW_OF_HEAD[h]) * QB,
                        ap=[[0, D], [1, QB]],
                    )
                    nc.sync.dma_start(out=rbc[D * h : D * h + D, :], in_=ri_b)
                attn2 = nrm.tile([E, QB], bf16, tag="attn2", name=f"attn2_{qb}")
                with nc.allow_low_precision(reason="softmax normalize in bf16"):
                    for h in range(H):
                        nc.gpsimd.tensor_tensor(
                            attn2[D * h : D * h + D, :],
                            attnT[D * h : D * h + D, :],
                            rbc[D * h : D * h + D, :],
                            op=mybir.AluOpType.mult,
                        )
                return attn2

            def emit_proj(qb, attnT):
                """project + store query block qb"""
                q0 = qb * QB
                pp = pssc.tile([E, 2 * QB], f32, tag="scps", name=f"pp{qb}")
                for m in range(QB // 128):
                    nc.tensor.matmul(
                        pp[:, 128 * m : 128 * m + 128],
                        attnT[:, 128 * m : 128 * m + 128],
                        wps_s,
                        start=(m == 0),
                        stop=(m == QB // 128 - 1),
                        skip_group_check=True,
                    )
                ob = outp.tile([E, QB], f32, tag="ob", name=f"ob{qb}")
                bp_rep = bass.AP(
                    tensor=bp_s[:].tensor,
                    offset=bp_s[:].offset,
                    ap=[list(bp_s[:].ap[0]), [0, QB // 128], [1, E]],
                )
                ob_v = ob[:].rearrange("p (m e) -> p m e", e=E)
                pp_v = pp[:, :QB].rearrange("p (m e) -> p m e", e=E)
                nc.vector.tensor_add(ob_v, pp_v, bp_rep)
                # single 3D DMA: ob [q=128, m, e] -> out rows q0+128m+p
                ob_dram = bass.AP(
                    tensor=out.ap().tensor,
                    offset=out.ap().offset + q0 * E,
                    ap=[[E, 128], [128 * E, QB // 128], [1, E]],
                )
                nc.sync.dma_start(out=ob_dram, in_=ob[:].rearrange("p (m e) -> p m e", e=E))

            NSC = NQ // QB * NKB * 2  # 256 (qb, chunk, pair) units

            def sc_tile(jj):
                return pssc.tile([E, 2 * QB], f32, tag="scps", name=f"sc{jj}")

            def emit_sc(jj, sc):
                qb, c, p = jj // (2 * NKB), (jj % (2 * NKB)) // 2, jj % 2
                q0, k0 = qb * QB, 128 * c
                for hh in range(2):
                    h = 2 * p + hh
                    nc.tensor.matmul(
                        sc[:, QB * hh : QB * hh + QB],
                        kT[D * h : D * h + D, k0 : k0 + 128],
                        qT[D * h : D * h + D, q0 : q0 + QB],
                        start=True,
                        stop=True,
                        tile_position=(D * h, 0),
                    )

            def emit_exp(jj, sc):
                """one exp unit: sc [E,1024] -> ex (bf16 view)"""
                use_dve = use_dve_for(jj)
                if use_dve:
                    ex = expool.tile([E, 2 * QB], i16, tag="ex", name=f"ex{jj}")
                    nc.vector.tensor_scalar(
                        ex[:],
                        sc[:],
                        SCHR_A,
                        SCHR_B,
                        op0=mybir.AluOpType.mult,
                        op1=mybir.AluOpType.add,
                    )
                    return ex[:].bitcast(mybir.dt.bfloat16)
                ex = expool.tile(
                    [E, 2 * QB], mybir.dt.bfloat16, tag="ex", name=f"ex{jj}"
                )
                nc.scalar.activation(
                    out=ex[:],
                    in_=sc[:],
                    func=mybir.ActivationFunctionType.Exp,
                    scale=SCALE,
                )
                return ex[:]

            def emit_pv(att_ps, c, exvs):
                for h in range(H):
                    nc.tensor.matmul(
                        att_slice(att_ps, h, slice(0, D + 1)),
                        v1[:, c, h, :],
                        exvs[h // 2][:, QB * (h % 2) : QB * (h % 2) + QB],
                        start=(c == 0),
                        stop=(c == NKB - 1),
                        tile_position=(0, 64 * (h % 2)),
                        skip_group_check=True,
                    )

            # per (qb, c): 4 score matmuls (chunk c+1), 2 exp units (chunk c),
            # 4 PV matmuls (chunk c) -- large same-kind PE groups
            NCU = NQ // QB * NKB  # 128 (qb, chunk) units
            att_tiles = {}
            pending_proj = []
            scs = {}

            def emit_scores(u):
                qb, c = u // NKB, u % NKB
                for p in range(2):
                    jj = 2 * u + p
                    scs[jj] = sc_tile(jj)
                    emit_sc(jj, scs[jj])

            # PV lags scores by one unit: PE never waits on the current
            # unit's exp, only the previous one's (already drained)
            emit_scores(0)
            exq = {}  # u -> exvs
            for u in range(NCU + 1):
                if u < NCU:
                    qb, c = u // NKB, u % NKB
                    if qb == 0:
                        # stream the kv projections under the first query block
                        if c % 4 == 0 and (c // 4 + 1) < N // QB:
                            emit_kT_chunk(c // 4 + 1)
                        emit_v_chunk(c)
                    if u + 1 < NCU:
                        emit_scores(u + 1)
                up = u - 1
                boundary = u >= 1 and (up % NKB) == NKB - 1
                if u < NCU and not boundary:
                    exq[u] = [emit_exp(2 * u + p, scs.pop(2 * u + p)) for p in range(2)]
                if u >= 1:
                    pqb, pc = up // NKB, up % NKB
                    if pc == 0:
                        att_tiles[pqb] = psatt.tile(
                            [E, 2 * QB], f32, tag="attps", name=f"attp{pqb}"
                        )
                    emit_pv(att_tiles[pqb], pc, exq.pop(up))
                    if boundary:
                        attnT = emit_norm_a(pqb, att_tiles.pop(pqb))
                        if u < NCU:
                            exq[u] = [
                                emit_exp(2 * u + p, scs.pop(2 * u + p))
                                for p in range(2)
                            ]
                        pending_proj.append((pqb, emit_norm_b(pqb, attnT)))
                    if pc == 8 and pending_proj:
                        piq, pattnT = pending_proj.pop(0)
                        emit_proj(piq, pattnT)
            while pending_proj:
                piq, pattnT = pending_proj.pop(0)
                emit_proj(piq, pattnT)

    if split:
        _split_multi_waits(nc)
    return nc


def _prep_host(x, W_qkv, b_qkv, W_proj, b_proj):
    j = np.arange(E)
    h, d = j // D, j % D
    cq = h * (3 * D) + d * 3 + 0
    ck = cq + 1
    cv = cq + 2
    Wq = W_qkv[:, cq].astype(np.float32)
    Wk = W_qkv[:, ck].astype(np.float32)
    Wv = W_qkv[:, cv].astype(np.float32)
    bq = b_qkv[cq].astype(np.float32)
    bk = b_qkv[ck].astype(np.float32)
    bv = b_qkv[cv].astype(np.float32)
    bp = (bv @ W_proj + b_proj).astype(np.float32).reshape(1, E)
    wpack = np.concatenate(
        [Wq, Wk, Wv, W_proj.astype(np.float32), bq.reshape(E, 1), bk.reshape(E, 1)],
        axis=1,
    )
    wpack = np.ascontiguousarray(wpack, np.float32)
    in_maps = []
    for c in range(NCORES):
        b, half = c // 2, c % 2
        xT_kv = np.ascontiguousarray(x[b].T).astype(ml_dtypes.bfloat16)
        xT_q = np.ascontiguousarray(
            x[b, half * NQ : (half + 1) * NQ].T
        ).astype(ml_dtypes.bfloat16)
        in_maps.append(
            {"xT_kv": xT_kv, "xT_q": xT_q, "wpack": wpack, "bp": bp}
        )
    return in_maps


def kernel(x, W_qkv, b_qkv, W_proj, b_proj, _trace=False):
    x = np.asarray(x, np.float32)
    W_qkv = np.asarray(W_qkv, np.float32)
    b_qkv = np.asarray(b_qkv, np.float32)
    W_proj = np.asarray(W_proj, np.float32)
    b_proj = np.asarray(b_proj, np.float32)

    from concourse.bass_utils import run_bass_kernel_spmd

    if "nc" not in _CACHE:
        _CACHE["nc"] = _build()
    nc = _CACHE["nc"]

    in_maps = _prep_host(x, W_qkv, b_qkv, W_proj, b_proj)
    res = run_bass_kernel_spmd(
        nc, in_maps, core_ids=list(range(NCORES)), trace=_trace
    )
    out = np.empty((B, N, E), np.float32)
    for c in range(NCORES):
        b, half = c // 2, c % 2
        out[b, half * NQ : (half + 1) * NQ] = res.results[c]["out"]
    if _trace:
        _CACHE["last_result"] = res
    return out
